# revision 21
# baseline (speedup 1.0000x reference)
"""Trainium2 Bass kernel for nn_ACCriticNSMsg (per-agent MLP critic with
message passing).

Math (per batch b, time t, agent a):
    inp   = concat(obs[b,t,a], messages[b,t-1,j != a])          # 128 + 112
    h     = relu(inp @ W1[a] + b1[a])                            # 240 -> 64
    q     = h @ W2[a] + b2[a]                                    # 64 -> 1

Reformulation: the "other agents' previous messages" gather is replaced by
the FULL 128-dim message vector of all 8 agents at t-1 multiplied against an
expanded weight matrix W1m[a] whose rows for agent a's own message slice are
zero.  Layer 1 becomes two K=128 matmuls accumulated in PSUM:

    h = relu(obs[b,t,a] @ W1o[a] + full_msg[b,t-1] @ W1m[a] + b1[a])

Since the msg rhs (msgT) is shared by all agents, the two agents of a pair
share ONE msg matmul with a [128, 128] stationary [W1m[a0] | W1m[a1]].

Distribution: pure data-parallel over the batch axis (bs=64 -> 8 cores x 8),
weights replicated; no collectives.

All layout transforms happen on the HOST: bf16 cast, message time-shift,
feature-major transpose of obs/msg, and the final un-transpose + b2 add of
the tiny (512 KB total) q output.  The device therefore does only plain
contiguous DMA at full HBM bandwidth and pure matmul/relu work.  (The
on-device DMA xbar transpose was measured to signal completion before its
data lands when all 8 cores contend for HBM, corrupting consumers — so
host-side transposes are also the *correct* choice, not just the fast one.)

Per-core dataflow (token columns pre-permuted to (agent, l=t%16, j=b*16+t//16)):
  - L1 per agent-pair: PSUM [128, 512] holds hT for 2 agents (rows 0:64 /
    64:128).  The paired msg matmul opens each bank (start=True, M=128 — a
    PSUM 'start' clears the zero-region flags across ALL partitions, so
    there must be exactly one start per bank), then 2 obs matmuls (M=64)
    accumulate.
  - relu+bias (ACT / DVE alternating) -> hT bf16 [128, 512].
  - L2: lhsT = [W2[a0]|W2[a1]|0...] [128, 32] -> q rows {32p, 32p+1} of a
    per-toktile shared PSUM bank (4 pairs col-packed, M=32 so all partitions
    are initialized).
  - q banks copied (cast bf16) straight into SBUF [128, 2048]; one DMA out.
"""

import numpy as np
import ml_dtypes

import concourse.bass as bass
import concourse.bacc as bacc
import concourse.tile as tile
from concourse import mybir
from concourse.bass_utils import run_bass_kernel_spmd
from concourse.bass_interp import get_hw_module

BF16 = mybir.dt.bfloat16
F32 = mybir.dt.float32
NPBF16 = ml_dtypes.bfloat16

# Problem shapes (hardcoded per spec)
BS, T, N, MSG_LEN, OBS_DIM, HID = 64, 256, 8, 16, 128, 64
N_CORES = 8
BS_LOC = BS // N_CORES          # 8 batches per core
TOK = BS_LOC * T                # 2048 (b, t) tokens per core (per agent)
TOK_ALL = N * TOK               # 16384 obs tokens per core
TT = 512                        # token tile (one PSUM bank of f32)
N_TT = TOK // TT                # 4 token tiles per agent
MSG_D = N * MSG_LEN             # 128 = full message vector
NP = N // 2                     # agent pairs


def _build():
    """Build the SPMD single-core graph (identical on all 8 cores)."""
    nc = bacc.Bacc("TRN2", target_bir_lowering=False, debug=False,
                   enable_asserts=False, num_devices=N_CORES)

    # ---- DRAM parameters (host pre-transposed/permuted, bf16) ----
    obs_d = nc.dram_tensor("obsT", [128, TOK_ALL], BF16, kind="ExternalInput").ap()
    msg_d = nc.dram_tensor("msgT", [128, TOK], BF16, kind="ExternalInput").ap()
    # bf16 blob: [128, w1o(8*64) + w1m(8*64) + w2(4*32)]
    CB = 2 * N * HID + 128
    cb_d = nc.dram_tensor("cblob16", [128, CB], BF16, kind="ExternalInput").ap()
    # f32 blob: [128, b1(4)]
    CF = NP
    cf_d = nc.dram_tensor("cblob32", [128, CF], F32, kind="ExternalInput").ap()
    # raw q output: [128, 2048] bf16; rows {32p, 32p+1} = agents (2p, 2p+1)
    out_d = nc.dram_tensor("out", [128, TOK], BF16, kind="ExternalOutput").ap()

    with tile.TileContext(nc) as tc:
        with tc.tile_pool(name="consts", bufs=1) as consts, \
             tc.tile_pool(name="acts", bufs=1) as acts, \
             tc.tile_pool(name="ht", bufs=4) as ht_pool, \
             tc.tile_pool(name="qsb", bufs=1) as qsb_pool, \
             tc.tile_pool(name="psl1", bufs=6, space="PSUM") as psl1, \
             tc.tile_pool(name="psl2", bufs=2, space="PSUM") as psl2:

            # ---- loads: single sync HWDGE queue, chunked so the first
            # matmuls' operands land early (blob leads with w1m) ----
            cb_sb = consts.tile([128, CB], BF16, tag="cb")
            nc.sync.dma_start(out=cb_sb[:, 0:2 * 128], in_=cb_d[:, 0:2 * 128])
            msgT = acts.tile([128, TOK], BF16, tag="msgT")
            nc.sync.dma_start(out=msgT[:, 0:TT], in_=msg_d[:, 0:TT])
            nc.sync.dma_start(out=cb_sb[:, 2 * 128:], in_=cb_d[:, 2 * 128:])
            cf_sb = consts.tile([128, CF], F32, tag="cf")
            nc.sync.dma_start(out=cf_sb[:], in_=cf_d)
            # obs is (toktile, agent)-major on the host; two 512 KB DMAs
            # per toktile so the first pairs' operands land sooner
            obsT = acts.tile([128, N_TT, N, TT], BF16, tag="obsT")
            obs_r = obs_d.rearrange("k (t g x) -> k t g x", t=N_TT, g=2)
            nc.sync.dma_start(out=obsT[:, 0, 0:4], in_=obs_r[:, 0, 0])
            nc.sync.dma_start(out=obsT[:, 0, 4:8], in_=obs_r[:, 0, 1])
            for t in range(1, N_TT):
                nc.sync.dma_start(out=msgT[:, bass.ts(t, TT)],
                                  in_=msg_d[:, bass.ts(t, TT)])
                nc.sync.dma_start(out=obsT[:, t, 0:4], in_=obs_r[:, t, 0])
                nc.sync.dma_start(out=obsT[:, t, 4:8], in_=obs_r[:, t, 1])
            # blob layout: [w1m(8*64) | w1o(8*64) | w2(4*32)]
            w1m_sb = cb_sb[:, 0:N * HID].rearrange(
                "k (p h) -> k p h", p=NP)          # [128, 4, 128] pair-major
            w1o_sb = cb_sb[:, N * HID:2 * N * HID].rearrange(
                "k (a h) -> k a h", a=N)
            w2_sb = cb_sb[:, 2 * N * HID:].rearrange(
                "k (p m) -> k p m", p=NP)          # [128, 4, 32]
            b1_sb = cf_sb[:, 0:NP]

            # raw q staging [128, 2048] bf16
            q_sb = qsb_pool.tile([128, TOK], BF16, tag="q_sb")

            for t in range(N_TT):             # token tiles (outer: lets the
                sl = bass.ts(t, TT)           # output stream out early)
                q_ps = psl2.tile([128, TT], F32, tag="q", name=f"q_ps{t}")
                # msg matmuls for all 4 pairs first (need only msgT chunk t):
                # each opens its bank with the single start=True write
                pss = []
                for p in range(NP):
                    ps = psl1.tile([128, TT], F32, tag="l1", name=f"l1_{p}_{t}")
                    nc.tensor.matmul(ps[:, :], lhsT=w1m_sb[:, p, :],
                                     rhs=msgT[:, sl],
                                     start=True, stop=False,
                                     skip_group_check=True)
                    pss.append(ps)
                for p in range(NP):           # agent pairs
                    a0, a1 = 2 * p, 2 * p + 1
                    ps = pss[p]
                    nc.tensor.matmul(ps[0:HID, :], lhsT=w1o_sb[:, a0, :],
                                     rhs=obsT[:, t, a0, :],
                                     start=False, stop=False,
                                     skip_group_check=True)
                    nc.tensor.matmul(ps[HID:128, :], lhsT=w1o_sb[:, a1, :],
                                     rhs=obsT[:, t, a1, :],
                                     start=False, stop=True,
                                     skip_group_check=True)
                    ht = ht_pool.tile([128, TT], BF16, tag="ht",
                                      name=f"ht_{p}_{t}")
                    if (t * NP + p) % 2 == 0:
                        nc.scalar.activation(out=ht[:], in_=ps[:],
                                             func=mybir.ActivationFunctionType.Relu,
                                             bias=b1_sb[:, p:p + 1], scale=1.0)
                    else:
                        nc.vector.tensor_scalar(out=ht[:], in0=ps[:],
                                                scalar1=b1_sb[:, p:p + 1],
                                                scalar2=0.0,
                                                op0=mybir.AluOpType.add,
                                                op1=mybir.AluOpType.max)
                    # layer 2: M=32 (cols 2..31 zero) so the 4 pairs jointly
                    # initialize every partition of the shared q bank
                    nc.tensor.matmul(q_ps[32 * p:32 * (p + 1), :],
                                     lhsT=w2_sb[:, p, :],
                                     rhs=ht[:], start=True, stop=True,
                                     tile_position=(0, 32 * p))
                # stream this toktile's q out (copy cast bf16 + chunk DMA)
                nc.vector.tensor_copy(out=q_sb[:, sl], in_=q_ps[:])
                nc.sync.dma_start(out=out_d[:, sl], in_=q_sb[:, sl])

    nc.compile()
    nc.m = get_hw_module(nc.m)
    return nc


_NC_CACHE = None


def _get_nc():
    global _NC_CACHE
    if _NC_CACHE is None:
        _NC_CACHE = _build()
    return _NC_CACHE


def _prep_inputs(obs, messages, W1, b1, W2, b2):
    """Host-side shard + repack + transpose. Returns in_maps for 8 cores."""
    obs = np.asarray(obs, dtype=np.float32)
    messages = np.asarray(messages, dtype=np.float32)
    W1 = np.asarray(W1, dtype=np.float32)
    b1 = np.asarray(b1, dtype=np.float32)
    W2 = np.asarray(W2, dtype=np.float32)

    # expanded message weights (own-agent slice zeroed), matching reference's
    # [prev agents, next agents] concat order
    W1o = W1[:, :OBS_DIM, :]                         # [8, 128, 64]
    W1m = np.zeros((N, MSG_D, HID), np.float32)
    for a in range(N):
        k = 0
        for j in range(N):
            if j == a:
                continue
            W1m[a, j * MSG_LEN:(j + 1) * MSG_LEN] = \
                W1[a, OBS_DIM + k * MSG_LEN: OBS_DIM + (k + 1) * MSG_LEN]
            k += 1

    w1o_k = W1o.transpose(1, 0, 2).reshape(128, N * HID)
    w1m_k = W1m.transpose(1, 0, 2).reshape(128, N * HID)
    w2p = np.zeros((128, NP, 32), np.float32)
    for p in range(NP):
        w2p[0:HID, p, 0] = W2[2 * p, :, 0]
        w2p[HID:128, p, 1] = W2[2 * p + 1, :, 0]
    w2p = w2p.reshape(128, NP * 32)
    cblob16 = np.concatenate([w1m_k, w1o_k, w2p], axis=1).astype(NPBF16)

    b1p = np.zeros((128, NP), np.float32)
    for p in range(NP):
        b1p[0:HID, p] = b1[2 * p]
        b1p[HID:128, p] = b1[2 * p + 1]
    cblob32 = b1p.astype(np.float32)

    # shifted full message vector [bs, T, 128]
    msgf = messages.reshape(BS, T, MSG_D)
    msgs_shift = np.zeros_like(msgf)
    msgs_shift[:, 1:] = msgf[:, :-1]

    in_maps = []
    for c in range(N_CORES):
        bsl = slice(c * BS_LOC, (c + 1) * BS_LOC)
        # token order per agent: (l=t%16, j=b*16 + t//16); obs columns
        # grouped (toktile, agent, 512)
        ob = obs[bsl].reshape(BS_LOC, 16, 16, N, OBS_DIM)
        o = ob.transpose(4, 3, 2, 0, 1).reshape(128, N, N_TT, TT)
        o = np.ascontiguousarray(o.transpose(0, 2, 1, 3)).reshape(
            128, TOK_ALL).astype(NPBF16)
        mb = msgs_shift[bsl].reshape(BS_LOC, 16, 16, MSG_D)
        m = np.ascontiguousarray(mb.transpose(3, 2, 0, 1)).reshape(
            128, TOK).astype(NPBF16)
        in_maps.append({
            "obsT": o, "msgT": m, "cblob16": cblob16, "cblob32": cblob32,
        })
    return in_maps


def _install_profile_hook():
    """The boot environment lacks antenv.axon_hooks; install the NTFF hook ourselves."""
    import sys as _sys
    import types as _types
    try:
        from antenv.axon_hooks import get_axon_ntff_profile_hook  # noqa: F401
        return
    except ImportError:
        pass
    try:
        import antenv
        from trn_agent_boot.trn_boot import _ntff_profile_via_ctypes
        hook = _ntff_profile_via_ctypes("/opt/axon/libaxon_pjrt.so")
        mod = _types.ModuleType("antenv.axon_hooks")
        mod._hook = hook
        mod.get_axon_ntff_profile_hook = lambda: mod._hook

        def _set(h):
            mod._hook = h

        mod.set_axon_ntff_profile_hook = _set
        _sys.modules["antenv.axon_hooks"] = mod
        antenv.axon_hooks = mod
    except Exception as e:  # profiling is best-effort
        print(f"profile hook install failed: {e}")


_ROWS = np.array([32 * (a // 2) + a % 2 for a in range(N)])


def run(obs, messages, W1, b1, W2, b2, trace=False):
    if trace:
        _install_profile_hook()
    nc = _get_nc()
    in_maps = _prep_inputs(obs, messages, W1, b1, W2, b2)
    res = run_bass_kernel_spmd(nc, in_maps, core_ids=list(range(N_CORES)),
                               trace=trace)
    b2 = np.asarray(b2, dtype=np.float32)
    outs = []
    for c in range(N_CORES):
        o = np.asarray(res.results[c]["out"]).astype(np.float32)  # [128, 2048]
        # rows 32p+i = agent 2p+i; cols (l, j) with j = b*16 + t_hi,
        # t = t_hi*16 + l
        qa = o[_ROWS].reshape(N, 16, BS_LOC, 16)       # [a, l, b, t_hi]
        q = qa.transpose(2, 3, 1, 0)                   # [b, t_hi, l, a]
        q = q.reshape(BS_LOC, T, N, 1) + b2[None, None, :, :]
        outs.append(q)
    full = np.concatenate(outs, axis=0).astype(np.float32)
    return full, res


def kernel(obs, messages, W1, b1, W2, b2):
    out, _ = run(obs, messages, W1, b1, W2, b2, trace=False)
    return out


# revision 22
# speedup vs baseline: 1.0016x; 1.0016x over previous
"""Trainium2 Bass kernel for nn_ACCriticNSMsg (per-agent MLP critic with
message passing).

Math (per batch b, time t, agent a):
    inp   = concat(obs[b,t,a], messages[b,t-1,j != a])          # 128 + 112
    h     = relu(inp @ W1[a] + b1[a])                            # 240 -> 64
    q     = h @ W2[a] + b2[a]                                    # 64 -> 1

Reformulation: the "other agents' previous messages" gather is replaced by
the FULL 128-dim message vector of all 8 agents at t-1 multiplied against an
expanded weight matrix W1m[a] whose rows for agent a's own message slice are
zero.  Layer 1 becomes two K=128 matmuls accumulated in PSUM:

    h = relu(obs[b,t,a] @ W1o[a] + full_msg[b,t-1] @ W1m[a] + b1[a])

Since the msg rhs (msgT) is shared by all agents, the two agents of a pair
share ONE msg matmul with a [128, 128] stationary [W1m[a0] | W1m[a1]].

Distribution: pure data-parallel over the batch axis (bs=64 -> 8 cores x 8),
weights replicated; no collectives.

All layout transforms happen on the HOST: bf16 cast, message time-shift,
feature-major transpose of obs/msg, and the final un-transpose + b2 add of
the tiny (512 KB total) q output.  The device therefore does only plain
contiguous DMA at full HBM bandwidth and pure matmul/relu work.  (The
on-device DMA xbar transpose was measured to signal completion before its
data lands when all 8 cores contend for HBM, corrupting consumers — so
host-side transposes are also the *correct* choice, not just the fast one.)

Per-core dataflow (token columns pre-permuted to (agent, l=t%16, j=b*16+t//16)):
  - L1 per agent-pair: PSUM [128, 512] holds hT for 2 agents (rows 0:64 /
    64:128).  The paired msg matmul opens each bank (start=True, M=128 — a
    PSUM 'start' clears the zero-region flags across ALL partitions, so
    there must be exactly one start per bank), then 2 obs matmuls (M=64)
    accumulate.
  - relu+bias (ACT / DVE alternating) -> hT bf16 [128, 512].
  - L2: lhsT = [W2[a0]|W2[a1]|0...] [128, 32] -> q rows {32p, 32p+1} of a
    per-toktile shared PSUM bank (4 pairs col-packed, M=32 so all partitions
    are initialized).
  - q banks copied (cast bf16) straight into SBUF [128, 2048]; one DMA out.
"""

import numpy as np
import ml_dtypes

import concourse.bass as bass
import concourse.bacc as bacc
import concourse.tile as tile
from concourse import mybir
from concourse.bass_utils import run_bass_kernel_spmd
from concourse.bass_interp import get_hw_module

BF16 = mybir.dt.bfloat16
F32 = mybir.dt.float32
NPBF16 = ml_dtypes.bfloat16

# Problem shapes (hardcoded per spec)
BS, T, N, MSG_LEN, OBS_DIM, HID = 64, 256, 8, 16, 128, 64
N_CORES = 8
BS_LOC = BS // N_CORES          # 8 batches per core
TOK = BS_LOC * T                # 2048 (b, t) tokens per core (per agent)
TOK_ALL = N * TOK               # 16384 obs tokens per core
TT = 512                        # token tile (one PSUM bank of f32)
N_TT = TOK // TT                # 4 token tiles per agent
MSG_D = N * MSG_LEN             # 128 = full message vector
NP = N // 2                     # agent pairs


def _build():
    """Build the SPMD single-core graph (identical on all 8 cores)."""
    nc = bacc.Bacc("TRN2", target_bir_lowering=False, debug=False,
                   enable_asserts=False, num_devices=N_CORES)

    # ---- DRAM parameters (host pre-transposed/permuted, bf16) ----
    obs_d = nc.dram_tensor("obsT", [128, TOK_ALL], BF16, kind="ExternalInput").ap()
    msg_d = nc.dram_tensor("msgT", [128, TOK], BF16, kind="ExternalInput").ap()
    # bf16 blob: [128, w1o(8*64) + w1m(8*64) + w2(4*32)]
    CB = 2 * N * HID + 128
    cb_d = nc.dram_tensor("cblob16", [128, CB], BF16, kind="ExternalInput").ap()
    # f32 blob: [128, b1(4)]
    CF = NP
    cf_d = nc.dram_tensor("cblob32", [128, CF], F32, kind="ExternalInput").ap()
    # raw q output: [128, 2048] bf16; rows {32p, 32p+1} = agents (2p, 2p+1)
    out_d = nc.dram_tensor("out", [128, TOK], BF16, kind="ExternalOutput").ap()

    with tile.TileContext(nc) as tc:
        with tc.tile_pool(name="consts", bufs=1) as consts, \
             tc.tile_pool(name="acts", bufs=1) as acts, \
             tc.tile_pool(name="ht", bufs=4) as ht_pool, \
             tc.tile_pool(name="qsb", bufs=1) as qsb_pool, \
             tc.tile_pool(name="psl1", bufs=6, space="PSUM") as psl1, \
             tc.tile_pool(name="psl2", bufs=2, space="PSUM") as psl2:

            # ---- loads: single sync HWDGE queue, chunked so the first
            # matmuls' operands land early (blob leads with w1m) ----
            cb_sb = consts.tile([128, CB], BF16, tag="cb")
            nc.sync.dma_start(out=cb_sb[:, 0:2 * 128], in_=cb_d[:, 0:2 * 128])
            msgT = acts.tile([128, TOK], BF16, tag="msgT")
            nc.sync.dma_start(out=msgT[:, 0:TT], in_=msg_d[:, 0:TT])
            nc.sync.dma_start(out=cb_sb[:, 2 * 128:], in_=cb_d[:, 2 * 128:])
            cf_sb = consts.tile([128, CF], F32, tag="cf")
            nc.sync.dma_start(out=cf_sb[:], in_=cf_d)
            # obs is (toktile, agent)-major on the host; two 512 KB DMAs
            # per toktile so the first pairs' operands land sooner
            obsT = acts.tile([128, N_TT, N, TT], BF16, tag="obsT")
            obs_r = obs_d.rearrange("k (t g x) -> k t g x", t=N_TT, g=2)
            nc.sync.dma_start(out=obsT[:, 0, 0:4], in_=obs_r[:, 0, 0])
            nc.sync.dma_start(out=obsT[:, 0, 4:8], in_=obs_r[:, 0, 1])
            for t in range(1, N_TT):
                nc.sync.dma_start(out=msgT[:, bass.ts(t, TT)],
                                  in_=msg_d[:, bass.ts(t, TT)])
                nc.sync.dma_start(out=obsT[:, t, 0:4], in_=obs_r[:, t, 0])
                nc.sync.dma_start(out=obsT[:, t, 4:8], in_=obs_r[:, t, 1])
            # blob layout: [w1m(8*64) | w1o(8*64) | w2(4*32)]
            w1m_sb = cb_sb[:, 0:N * HID].rearrange(
                "k (p h) -> k p h", p=NP)          # [128, 4, 128] pair-major
            w1o_sb = cb_sb[:, N * HID:2 * N * HID].rearrange(
                "k (a h) -> k a h", a=N)
            w2_sb = cb_sb[:, 2 * N * HID:].rearrange(
                "k (p m) -> k p m", p=NP)          # [128, 4, 32]
            b1_sb = cf_sb[:, 0:NP]

            # raw q staging [128, 2048] bf16
            q_sb = qsb_pool.tile([128, TOK], BF16, tag="q_sb")

            for t in range(N_TT):             # token tiles (outer: lets the
                sl = bass.ts(t, TT)           # output stream out early)
                q_ps = psl2.tile([128, TT], F32, tag="q", name=f"q_ps{t}")
                for p in range(NP):           # agent pairs
                    a0, a1 = 2 * p, 2 * p + 1
                    ps = psl1.tile([128, TT], F32, tag="l1", name=f"l1_{p}_{t}")
                    # ONE start=True per bank: paired msg matmul opens all
                    # 128 rows, obs matmuls accumulate into their halves.
                    nc.tensor.matmul(ps[:, :], lhsT=w1m_sb[:, p, :],
                                     rhs=msgT[:, sl],
                                     start=True, stop=False,
                                     skip_group_check=True)
                    nc.tensor.matmul(ps[0:HID, :], lhsT=w1o_sb[:, a0, :],
                                     rhs=obsT[:, t, a0, :],
                                     start=False, stop=False,
                                     skip_group_check=True)
                    nc.tensor.matmul(ps[HID:128, :], lhsT=w1o_sb[:, a1, :],
                                     rhs=obsT[:, t, a1, :],
                                     start=False, stop=True,
                                     skip_group_check=True)
                    ht = ht_pool.tile([128, TT], BF16, tag="ht",
                                      name=f"ht_{p}_{t}")
                    if (t * NP + p) % 2 == 0:
                        nc.scalar.activation(out=ht[:], in_=ps[:],
                                             func=mybir.ActivationFunctionType.Relu,
                                             bias=b1_sb[:, p:p + 1], scale=1.0)
                    else:
                        nc.vector.tensor_scalar(out=ht[:], in0=ps[:],
                                                scalar1=b1_sb[:, p:p + 1],
                                                scalar2=0.0,
                                                op0=mybir.AluOpType.add,
                                                op1=mybir.AluOpType.max)
                    # layer 2: M=32 (cols 2..31 zero) so the 4 pairs jointly
                    # initialize every partition of the shared q bank
                    nc.tensor.matmul(q_ps[32 * p:32 * (p + 1), :],
                                     lhsT=w2_sb[:, p, :],
                                     rhs=ht[:], start=True, stop=True,
                                     tile_position=(0, 32 * p))
                # stream this toktile's q out (copy cast bf16 + chunk DMA)
                nc.vector.tensor_copy(out=q_sb[:, sl], in_=q_ps[:])
                nc.sync.dma_start(out=out_d[:, sl], in_=q_sb[:, sl])

    nc.compile()
    nc.m = get_hw_module(nc.m)
    return nc


_NC_CACHE = None


def _get_nc():
    global _NC_CACHE
    if _NC_CACHE is None:
        _NC_CACHE = _build()
    return _NC_CACHE


def _prep_inputs(obs, messages, W1, b1, W2, b2):
    """Host-side shard + repack + transpose. Returns in_maps for 8 cores."""
    obs = np.asarray(obs, dtype=np.float32)
    messages = np.asarray(messages, dtype=np.float32)
    W1 = np.asarray(W1, dtype=np.float32)
    b1 = np.asarray(b1, dtype=np.float32)
    W2 = np.asarray(W2, dtype=np.float32)

    # expanded message weights (own-agent slice zeroed), matching reference's
    # [prev agents, next agents] concat order
    W1o = W1[:, :OBS_DIM, :]                         # [8, 128, 64]
    W1m = np.zeros((N, MSG_D, HID), np.float32)
    for a in range(N):
        k = 0
        for j in range(N):
            if j == a:
                continue
            W1m[a, j * MSG_LEN:(j + 1) * MSG_LEN] = \
                W1[a, OBS_DIM + k * MSG_LEN: OBS_DIM + (k + 1) * MSG_LEN]
            k += 1

    w1o_k = W1o.transpose(1, 0, 2).reshape(128, N * HID)
    w1m_k = W1m.transpose(1, 0, 2).reshape(128, N * HID)
    w2p = np.zeros((128, NP, 32), np.float32)
    for p in range(NP):
        w2p[0:HID, p, 0] = W2[2 * p, :, 0]
        w2p[HID:128, p, 1] = W2[2 * p + 1, :, 0]
    w2p = w2p.reshape(128, NP * 32)
    cblob16 = np.concatenate([w1m_k, w1o_k, w2p], axis=1).astype(NPBF16)

    b1p = np.zeros((128, NP), np.float32)
    for p in range(NP):
        b1p[0:HID, p] = b1[2 * p]
        b1p[HID:128, p] = b1[2 * p + 1]
    cblob32 = b1p.astype(np.float32)

    # shifted full message vector [bs, T, 128]
    msgf = messages.reshape(BS, T, MSG_D)
    msgs_shift = np.zeros_like(msgf)
    msgs_shift[:, 1:] = msgf[:, :-1]

    in_maps = []
    for c in range(N_CORES):
        bsl = slice(c * BS_LOC, (c + 1) * BS_LOC)
        # token order per agent: (l=t%16, j=b*16 + t//16); obs columns
        # grouped (toktile, agent, 512)
        ob = obs[bsl].reshape(BS_LOC, 16, 16, N, OBS_DIM)
        o = ob.transpose(4, 3, 2, 0, 1).reshape(128, N, N_TT, TT)
        o = np.ascontiguousarray(o.transpose(0, 2, 1, 3)).reshape(
            128, TOK_ALL).astype(NPBF16)
        mb = msgs_shift[bsl].reshape(BS_LOC, 16, 16, MSG_D)
        m = np.ascontiguousarray(mb.transpose(3, 2, 0, 1)).reshape(
            128, TOK).astype(NPBF16)
        in_maps.append({
            "obsT": o, "msgT": m, "cblob16": cblob16, "cblob32": cblob32,
        })
    return in_maps


def _install_profile_hook():
    """The boot environment lacks antenv.axon_hooks; install the NTFF hook ourselves."""
    import sys as _sys
    import types as _types
    try:
        from antenv.axon_hooks import get_axon_ntff_profile_hook  # noqa: F401
        return
    except ImportError:
        pass
    try:
        import antenv
        from trn_agent_boot.trn_boot import _ntff_profile_via_ctypes
        hook = _ntff_profile_via_ctypes("/opt/axon/libaxon_pjrt.so")
        mod = _types.ModuleType("antenv.axon_hooks")
        mod._hook = hook
        mod.get_axon_ntff_profile_hook = lambda: mod._hook

        def _set(h):
            mod._hook = h

        mod.set_axon_ntff_profile_hook = _set
        _sys.modules["antenv.axon_hooks"] = mod
        antenv.axon_hooks = mod
    except Exception as e:  # profiling is best-effort
        print(f"profile hook install failed: {e}")


_ROWS = np.array([32 * (a // 2) + a % 2 for a in range(N)])


def run(obs, messages, W1, b1, W2, b2, trace=False):
    if trace:
        _install_profile_hook()
    nc = _get_nc()
    in_maps = _prep_inputs(obs, messages, W1, b1, W2, b2)
    res = run_bass_kernel_spmd(nc, in_maps, core_ids=list(range(N_CORES)),
                               trace=trace)
    b2 = np.asarray(b2, dtype=np.float32)
    outs = []
    for c in range(N_CORES):
        o = np.asarray(res.results[c]["out"]).astype(np.float32)  # [128, 2048]
        # rows 32p+i = agent 2p+i; cols (l, j) with j = b*16 + t_hi,
        # t = t_hi*16 + l
        qa = o[_ROWS].reshape(N, 16, BS_LOC, 16)       # [a, l, b, t_hi]
        q = qa.transpose(2, 3, 1, 0)                   # [b, t_hi, l, a]
        q = q.reshape(BS_LOC, T, N, 1) + b2[None, None, :, :]
        outs.append(q)
    full = np.concatenate(outs, axis=0).astype(np.float32)
    return full, res


def kernel(obs, messages, W1, b1, W2, b2):
    out, _ = run(obs, messages, W1, b1, W2, b2, trace=False)
    return out


# revision 23
# speedup vs baseline: 1.0720x; 1.0702x over previous
"""Trainium2 Bass kernel for nn_ACCriticNSMsg (per-agent MLP critic with
message passing).

Math (per batch b, time t, agent a):
    inp   = concat(obs[b,t,a], messages[b,t-1,j != a])          # 128 + 112
    h     = relu(inp @ W1[a] + b1[a])                            # 240 -> 64
    q     = h @ W2[a] + b2[a]                                    # 64 -> 1

Reformulation: the "other agents' previous messages" gather is replaced by
the FULL 128-dim message vector of all 8 agents at t-1 multiplied against an
expanded weight matrix W1m[a] whose rows for agent a's own message slice are
zero.  Layer 1 becomes two K=128 matmuls accumulated in PSUM:

    h = relu(obs[b,t,a] @ W1o[a] + full_msg[b,t-1] @ W1m[a] + b1[a])

Since the msg rhs (msgT) is shared by all agents, the two agents of a pair
share ONE msg matmul with a [128, 128] stationary [W1m[a0] | W1m[a1]].

Distribution: pure data-parallel over the batch axis (bs=64 -> 8 cores x 8),
weights replicated; no collectives.

All layout transforms happen on the HOST: bf16 cast, message time-shift,
feature-major transpose of obs/msg, and the final un-transpose + b2 add of
the tiny (512 KB total) q output.  The device therefore does only plain
contiguous DMA at full HBM bandwidth and pure matmul/relu work.  (The
on-device DMA xbar transpose was measured to signal completion before its
data lands when all 8 cores contend for HBM, corrupting consumers — so
host-side transposes are also the *correct* choice, not just the fast one.)

Per-core dataflow (token columns pre-permuted to (agent, l=t%16, j=b*16+t//16)):
  - L1 per agent-pair: PSUM [128, 512] holds hT for 2 agents (rows 0:64 /
    64:128).  The paired msg matmul opens each bank (start=True, M=128 — a
    PSUM 'start' clears the zero-region flags across ALL partitions, so
    there must be exactly one start per bank), then 2 obs matmuls (M=64)
    accumulate.
  - relu+bias (ACT / DVE alternating) -> hT bf16 [128, 512].
  - L2: lhsT = [W2[a0]|W2[a1]|0...] [128, 32] -> q rows {32p, 32p+1} of a
    per-toktile shared PSUM bank (4 pairs col-packed, M=32 so all partitions
    are initialized).
  - q banks copied (cast bf16) straight into SBUF [128, 2048]; one DMA out.
"""

import numpy as np
import ml_dtypes

import concourse.bass as bass
import concourse.bacc as bacc
import concourse.tile as tile
from concourse import mybir
from concourse.bass_utils import run_bass_kernel_spmd
from concourse.bass_interp import get_hw_module

BF16 = mybir.dt.bfloat16
F32 = mybir.dt.float32
NPBF16 = ml_dtypes.bfloat16

# Problem shapes (hardcoded per spec)
BS, T, N, MSG_LEN, OBS_DIM, HID = 64, 256, 8, 16, 128, 64
N_CORES = 8
BS_LOC = BS // N_CORES          # 8 batches per core
TOK = BS_LOC * T                # 2048 (b, t) tokens per core (per agent)
TOK_ALL = N * TOK               # 16384 obs tokens per core
TT = 512                        # token tile (one PSUM bank of f32)
N_TT = TOK // TT                # 4 token tiles per agent
MSG_D = N * MSG_LEN             # 128 = full message vector
NP = N // 2                     # agent pairs


def _build():
    """Build the SPMD single-core graph (identical on all 8 cores)."""
    nc = bacc.Bacc("TRN2", target_bir_lowering=False, debug=False,
                   enable_asserts=False, num_devices=N_CORES)

    # ---- DRAM parameters (host pre-transposed/permuted, bf16) ----
    obs_d = nc.dram_tensor("obsT", [128, TOK_ALL], BF16, kind="ExternalInput").ap()
    msg_d = nc.dram_tensor("msgT", [128, TOK], BF16, kind="ExternalInput").ap()
    # bf16 blob: [128, w1o(8*64) + w1m(8*64) + w2(4*32)]
    CB = 2 * N * HID + 128
    cb_d = nc.dram_tensor("cblob16", [128, CB], BF16, kind="ExternalInput").ap()
    # f32 blob: [128, b1(4)]
    CF = NP
    cf_d = nc.dram_tensor("cblob32", [128, CF], F32, kind="ExternalInput").ap()
    # raw q output: [128, 2048] bf16; rows {32p, 32p+1} = agents (2p, 2p+1)
    out_d = nc.dram_tensor("out", [128, TOK], BF16, kind="ExternalOutput").ap()

    with tile.TileContext(nc) as tc:
        with tc.tile_pool(name="consts", bufs=1) as consts, \
             tc.tile_pool(name="acts", bufs=1) as acts, \
             tc.tile_pool(name="ht", bufs=4) as ht_pool, \
             tc.tile_pool(name="qsb", bufs=1) as qsb_pool, \
             tc.tile_pool(name="psl1", bufs=6, space="PSUM") as psl1, \
             tc.tile_pool(name="psl2", bufs=2, space="PSUM") as psl2:

            # ---- loads: single sync HWDGE queue, chunked so the first
            # matmuls' operands land early (blob leads with w1m) ----
            cb_sb = consts.tile([128, CB], BF16, tag="cb")
            nc.sync.dma_start(out=cb_sb[:, 0:2 * 128], in_=cb_d[:, 0:2 * 128])
            msgT = acts.tile([128, TOK], BF16, tag="msgT")
            nc.sync.dma_start(out=msgT[:, 0:TT], in_=msg_d[:, 0:TT])
            nc.sync.dma_start(out=cb_sb[:, 2 * 128:], in_=cb_d[:, 2 * 128:])
            cf_sb = consts.tile([128, CF], F32, tag="cf")
            nc.sync.dma_start(out=cf_sb[:], in_=cf_d)
            # obs is (toktile, agent)-major on the host: one 1 MB DMA
            # delivers a whole toktile for all 8 agents
            obsT = acts.tile([128, N_TT, N, TT], BF16, tag="obsT")
            obs_r = obs_d.rearrange("k (t x) -> k t x", t=N_TT)
            nc.sync.dma_start(out=obsT[:, 0], in_=obs_r[:, 0])
            for t in range(1, N_TT):
                nc.sync.dma_start(out=msgT[:, bass.ts(t, TT)],
                                  in_=msg_d[:, bass.ts(t, TT)])
                nc.sync.dma_start(out=obsT[:, t], in_=obs_r[:, t])
            # blob layout: [w1m(8*64) | w1o(8*64) | w2(4*32)]
            w1m_sb = cb_sb[:, 0:N * HID].rearrange(
                "k (p h) -> k p h", p=NP)          # [128, 4, 128] pair-major
            w1o_sb = cb_sb[:, N * HID:2 * N * HID].rearrange(
                "k (a h) -> k a h", a=N)
            w2_sb = cb_sb[:, 2 * N * HID:].rearrange(
                "k (p m) -> k p m", p=NP)          # [128, 4, 32]
            b1_sb = cf_sb[:, 0:NP]

            # raw q staging [128, 2048] bf16
            q_sb = qsb_pool.tile([128, TOK], BF16, tag="q_sb")

            for t in range(N_TT):             # token tiles (outer: lets the
                sl = bass.ts(t, TT)           # output stream out early)
                q_ps = psl2.tile([128, TT], F32, tag="q", name=f"q_ps{t}")
                for p in range(NP):           # agent pairs
                    a0, a1 = 2 * p, 2 * p + 1
                    ps = psl1.tile([128, TT], F32, tag="l1", name=f"l1_{p}_{t}")
                    # ONE start=True per bank: paired msg matmul opens all
                    # 128 rows, obs matmuls accumulate into their halves.
                    nc.tensor.matmul(ps[:, :], lhsT=w1m_sb[:, p, :],
                                     rhs=msgT[:, sl],
                                     start=True, stop=False,
                                     skip_group_check=True)
                    nc.tensor.matmul(ps[0:HID, :], lhsT=w1o_sb[:, a0, :],
                                     rhs=obsT[:, t, a0, :],
                                     start=False, stop=False,
                                     skip_group_check=True)
                    nc.tensor.matmul(ps[HID:128, :], lhsT=w1o_sb[:, a1, :],
                                     rhs=obsT[:, t, a1, :],
                                     start=False, stop=True,
                                     skip_group_check=True)
                    ht = ht_pool.tile([128, TT], BF16, tag="ht",
                                      name=f"ht_{p}_{t}")
                    if (t * NP + p) % 2 == 0:
                        nc.scalar.activation(out=ht[:], in_=ps[:],
                                             func=mybir.ActivationFunctionType.Relu,
                                             bias=b1_sb[:, p:p + 1], scale=1.0)
                    else:
                        nc.vector.tensor_scalar(out=ht[:], in0=ps[:],
                                                scalar1=b1_sb[:, p:p + 1],
                                                scalar2=0.0,
                                                op0=mybir.AluOpType.add,
                                                op1=mybir.AluOpType.max)
                    # layer 2: M=32 (cols 2..31 zero) so the 4 pairs jointly
                    # initialize every partition of the shared q bank
                    nc.tensor.matmul(q_ps[32 * p:32 * (p + 1), :],
                                     lhsT=w2_sb[:, p, :],
                                     rhs=ht[:], start=True, stop=True,
                                     tile_position=(0, 32 * p))
                # stream this toktile's q out (copy cast bf16 + chunk DMA)
                nc.vector.tensor_copy(out=q_sb[:, sl], in_=q_ps[:])
                nc.sync.dma_start(out=out_d[:, sl], in_=q_sb[:, sl])

    nc.compile()
    nc.m = get_hw_module(nc.m)
    return nc


_NC_CACHE = None


def _get_nc():
    global _NC_CACHE
    if _NC_CACHE is None:
        _NC_CACHE = _build()
    return _NC_CACHE


def _prep_inputs(obs, messages, W1, b1, W2, b2):
    """Host-side shard + repack + transpose. Returns in_maps for 8 cores."""
    obs = np.asarray(obs, dtype=np.float32)
    messages = np.asarray(messages, dtype=np.float32)
    W1 = np.asarray(W1, dtype=np.float32)
    b1 = np.asarray(b1, dtype=np.float32)
    W2 = np.asarray(W2, dtype=np.float32)

    # expanded message weights (own-agent slice zeroed), matching reference's
    # [prev agents, next agents] concat order
    W1o = W1[:, :OBS_DIM, :]                         # [8, 128, 64]
    W1m = np.zeros((N, MSG_D, HID), np.float32)
    for a in range(N):
        k = 0
        for j in range(N):
            if j == a:
                continue
            W1m[a, j * MSG_LEN:(j + 1) * MSG_LEN] = \
                W1[a, OBS_DIM + k * MSG_LEN: OBS_DIM + (k + 1) * MSG_LEN]
            k += 1

    w1o_k = W1o.transpose(1, 0, 2).reshape(128, N * HID)
    w1m_k = W1m.transpose(1, 0, 2).reshape(128, N * HID)
    w2p = np.zeros((128, NP, 32), np.float32)
    for p in range(NP):
        w2p[0:HID, p, 0] = W2[2 * p, :, 0]
        w2p[HID:128, p, 1] = W2[2 * p + 1, :, 0]
    w2p = w2p.reshape(128, NP * 32)
    cblob16 = np.concatenate([w1m_k, w1o_k, w2p], axis=1).astype(NPBF16)

    b1p = np.zeros((128, NP), np.float32)
    for p in range(NP):
        b1p[0:HID, p] = b1[2 * p]
        b1p[HID:128, p] = b1[2 * p + 1]
    cblob32 = b1p.astype(np.float32)

    # shifted full message vector [bs, T, 128]
    msgf = messages.reshape(BS, T, MSG_D)
    msgs_shift = np.zeros_like(msgf)
    msgs_shift[:, 1:] = msgf[:, :-1]

    in_maps = []
    for c in range(N_CORES):
        bsl = slice(c * BS_LOC, (c + 1) * BS_LOC)
        # token order per agent: (l=t%16, j=b*16 + t//16); obs columns
        # grouped (toktile, agent, 512)
        ob = obs[bsl].reshape(BS_LOC, 16, 16, N, OBS_DIM)
        o = ob.transpose(4, 3, 2, 0, 1).reshape(128, N, N_TT, TT)
        o = np.ascontiguousarray(o.transpose(0, 2, 1, 3)).reshape(
            128, TOK_ALL).astype(NPBF16)
        mb = msgs_shift[bsl].reshape(BS_LOC, 16, 16, MSG_D)
        m = np.ascontiguousarray(mb.transpose(3, 2, 0, 1)).reshape(
            128, TOK).astype(NPBF16)
        in_maps.append({
            "obsT": o, "msgT": m, "cblob16": cblob16, "cblob32": cblob32,
        })
    return in_maps


def _install_profile_hook():
    """The boot environment lacks antenv.axon_hooks; install the NTFF hook ourselves."""
    import sys as _sys
    import types as _types
    try:
        from antenv.axon_hooks import get_axon_ntff_profile_hook  # noqa: F401
        return
    except ImportError:
        pass
    try:
        import antenv
        from trn_agent_boot.trn_boot import _ntff_profile_via_ctypes
        hook = _ntff_profile_via_ctypes("/opt/axon/libaxon_pjrt.so")
        mod = _types.ModuleType("antenv.axon_hooks")
        mod._hook = hook
        mod.get_axon_ntff_profile_hook = lambda: mod._hook

        def _set(h):
            mod._hook = h

        mod.set_axon_ntff_profile_hook = _set
        _sys.modules["antenv.axon_hooks"] = mod
        antenv.axon_hooks = mod
    except Exception as e:  # profiling is best-effort
        print(f"profile hook install failed: {e}")


_ROWS = np.array([32 * (a // 2) + a % 2 for a in range(N)])


def run(obs, messages, W1, b1, W2, b2, trace=False):
    if trace:
        _install_profile_hook()
    nc = _get_nc()
    in_maps = _prep_inputs(obs, messages, W1, b1, W2, b2)
    res = run_bass_kernel_spmd(nc, in_maps, core_ids=list(range(N_CORES)),
                               trace=trace)
    b2 = np.asarray(b2, dtype=np.float32)
    outs = []
    for c in range(N_CORES):
        o = np.asarray(res.results[c]["out"]).astype(np.float32)  # [128, 2048]
        # rows 32p+i = agent 2p+i; cols (l, j) with j = b*16 + t_hi,
        # t = t_hi*16 + l
        qa = o[_ROWS].reshape(N, 16, BS_LOC, 16)       # [a, l, b, t_hi]
        q = qa.transpose(2, 3, 1, 0)                   # [b, t_hi, l, a]
        q = q.reshape(BS_LOC, T, N, 1) + b2[None, None, :, :]
        outs.append(q)
    full = np.concatenate(outs, axis=0).astype(np.float32)
    return full, res


def kernel(obs, messages, W1, b1, W2, b2):
    out, _ = run(obs, messages, W1, b1, W2, b2, trace=False)
    return out
